# revision 8
# baseline (speedup 1.0000x reference)
"""Trainium2 Bass kernel for nn_EquivariantDecoder (PaiNN-style equivariant GNN).

Strategy (8 NeuronCores, graph-parallel):
  - Host: filter edges with env==0 (dist >= cutoff, exact zero contribution),
    shard nodes contiguously across cores, sort each core's edges by
    destination, group into 128-node destination tiles / 128-edge chunks.
  - Device, per layer: AllGather of per-node [phi | V] rows (bf16) builds a
    global gather table in each core's HBM; per 128-edge chunk an indirect
    DMA gathers source rows; edge messages are elementwise work on
    DVE/ACT; segment-sums are one-hot matmuls accumulating in PSUM
    (destination-sorted edges keep scatters core-local). The cross-product
    term uses cross(V[i], .) distributing over the segment sum, so it is
    evaluated per node, not per edge. Update block + next layer's phi MLP
    run node-locally; the final head produces the outputs.
"""
import math
import numpy as np
import ml_dtypes

import concourse.bass as bass
import concourse.tile as tile
from concourse import mybir
from concourse.bass_utils import run_bass_kernel_spmd
from concourse.vector_clock import ScopedClock

# ---------------------------------------------------------------- constants
F = 78
F4 = 4 * F          # 312
NRBF = 20
CUTOFF = 21.0
L = 3
EPS = 1e-15
NCORE = 8
P = 128
BF = mybir.dt.bfloat16
FP = mybir.dt.float32
I32 = mybir.dt.int32
AF = mybir.ActivationFunctionType
OP = mybir.AluOpType

# ------------------------------------------------------- walrus workarounds
_PATCHED = False


def _apply_patches():
    """This walrus build accepts only ONE sync-wait per instruction.
    (a) patch TileContext's exit drain to spread its final-clock waits over
    single-wait SP nops; (b) post-pass that splits any instruction's extra
    waits onto same-engine nop carriers."""
    global _PATCHED
    if _PATCHED:
        return
    _PATCHED = True

    def _drain_and_barrier(self, tick_clock, wait_clock):
        nc = self.nc
        carrier = nc.sync.nop(nofuse=True, hint="drain_waits").ins
        wait_clock.add_sem_waits(carrier, ScopedClock({None: tick_clock.global_clock}))
        waits = list(carrier.sync_info.on_wait or [])
        if len(waits) > 1:
            carrier.sync_info.on_wait = waits[:1]
            for w in waits[1:]:
                c2 = nc.sync.nop(nofuse=True, hint="drain_waits").ins
                c2.sync_info = mybir.SyncInfo(on_wait=[w], on_update=[])
        nc.sync.drain()
        nc.all_engine_barrier()
        popped = nc._tile_sem_poison_stack.pop()
        assert popped is self._sem_poison
        nc.clear_and_free_semaphores(list(self.sems.allocated().values()))
        nc.all_engine_barrier()

    tile.TileContext._drain_and_barrier = _drain_and_barrier


_WS_CNT = [0]


def _split_multiwaits(nc):
    for f in nc.m.functions:
        for bb in f.blocks:
            out = []
            changed = False
            for inst in bb.instructions:
                si = getattr(inst, "sync_info", None)
                if si is not None and si.on_wait and len(si.on_wait) > 1:
                    waits = list(si.on_wait)
                    for w in waits[:-1]:
                        _WS_CNT[0] += 1
                        out.append(mybir.InstNoOp(
                            name=f"WS-{_WS_CNT[0]}-{inst.name}",
                            engine=inst.engine,
                            bass_nofuse=True,
                            sync_info=mybir.SyncInfo(on_wait=[w], on_update=[]),
                        ))
                    si.on_wait = waits[-1:]
                    changed = True
                out.append(inst)
            if changed:
                try:
                    bb.instructions[:] = out
                except TypeError:
                    bb.instructions = out


# -------------------------------------------------------------- device build
def _build_nc(n_nodes, nt, q):
    """One SPMD program for all 8 cores. nt destination tiles of 128 nodes per
    core, q chunks of 128 edges per tile."""
    ncols = nt * q                  # idx columns per core
    s_slots = ncols * P             # edge slots per core
    trow = 552                      # ptab row: [phi 312 | V 234 | pad 6] bf16
    nsh_pad = nt * P

    nc = bass.Bass()

    # ---- inputs
    inp = {}
    def din(name, shape, dt):
        inp[name] = nc.dram_tensor(name, shape, dt, kind="ExternalInput")
        return inp[name]

    idxj = din("idxj", [P, ncols], I32)
    iloc = din("iloc", [P, ncols], BF)
    rbft = din("rbft", [NRBF, s_slots], BF)
    envc = din("envc", [P, ncols], FP)
    unitc = din("unitc", [P, 3 * ncols], FP)
    hsh = din("hsh", [P, nt, F], FP)
    w1h = din("w1h", [F, L, F], BF);    b1h = din("b1h", [1, L, F], BF)
    w2h = din("w2h", [F, L, F4], BF);   b2h = din("b2h", [1, L, F4], BF)
    rwh = din("rwh", [NRBF, L, F4], BF); rbh = din("rbh", [1, L, F4], BF)
    wuh = din("wuh", [F, L, F], BF);    wvh = din("wvh", [F, L, F], BF)
    u1a = din("u1a", [F, L, F], BF);    u1b = din("u1b", [F, L, F], BF)
    ub1 = din("ub1", [1, L, F], BF)
    u2h = din("u2h", [F, L, 3 * F], BF); ub2 = din("ub2", [1, L, 3 * F], BF)
    dw1 = din("dw1", [F, 39], BF);      db1 = din("db1", [1, 39], BF)
    dw2 = din("dw2", [39, 39], BF);     db2 = din("db2", [1, 39], BF)
    onesr = din("onesr", [1, P], BF)
    iota = din("iota", [P, P], BF)
    ident = din("ident", [P, P], BF)

    h_out = nc.dram_tensor("h_out", [nsh_pad, F], FP, kind="ExternalOutput")
    y_out = nc.dram_tensor("y_out", [nsh_pad, 39], FP, kind="ExternalOutput")

    # internal DRAM: per-layer exchange buffers and gather tables
    nsh = n_nodes // NCORE
    pvx = [nc.dram_tensor(f"pvx{l}", [nsh, trow], BF) for l in range(L)]
    ptab = [nc.dram_tensor(f"ptab{l}", [NCORE * nsh, trow], BF,
                           addr_space="Shared") for l in range(L)]

    C = F * 3  # 234

    with tile.TileContext(nc) as tc:
        with (
            tc.tile_pool(name="const", bufs=1) as cp,
            tc.tile_pool(name="res", bufs=1) as rp,
            tc.tile_pool(name="work", bufs=3) as wp,
            tc.tile_pool(name="gat", bufs=4) as gp,
            tc.tile_pool(name="ep", bufs=2) as ep,
            tc.tile_pool(name="psA", bufs=1, space="PSUM") as psA,
            tc.tile_pool(name="psW", bufs=1, space="PSUM") as psW,
            tc.tile_pool(name="psE", bufs=2, space="PSUM") as psE,
        ):
            # ---------------- load constants / resident data
            def load(pool, src, shape, dt, tag):
                t = pool.tile(shape, dt, tag=tag)
                nc.sync.dma_start(out=t[:], in_=src[:])
                return t

            w1 = load(cp, w1h, [F, L, F], BF, "w1");   b1 = load(cp, b1h, [1, L, F], BF, "b1")
            w2 = load(cp, w2h, [F, L, F4], BF, "w2");  b2 = load(cp, b2h, [1, L, F4], BF, "b2")
            rw = load(cp, rwh, [NRBF, L, F4], BF, "rw"); rb = load(cp, rbh, [1, L, F4], BF, "rb")
            wu = load(cp, wuh, [F, L, F], BF, "wu");   wv = load(cp, wvh, [F, L, F], BF, "wv")
            w1a = load(cp, u1a, [F, L, F], BF, "w1a");  w1b = load(cp, u1b, [F, L, F], BF, "w1b")
            b1u = load(cp, ub1, [1, L, F], BF, "b1u")
            w2u = load(cp, u2h, [F, L, 3 * F], BF, "w2u"); b2u = load(cp, ub2, [1, L, 3 * F], BF, "b2u")
            hw1 = load(cp, dw1, [F, 39], BF, "hw1");    hb1 = load(cp, db1, [1, 39], BF, "hb1")
            hw2 = load(cp, dw2, [39, 39], BF, "hw2");   hb2 = load(cp, db2, [1, 39], BF, "hb2")
            ones = load(cp, onesr, [1, P], BF, "ones")
            iot = load(cp, iota, [P, P], BF, "iot")
            idn = load(cp, ident, [P, P], BF, "idn")

            idxt = load(rp, idxj, [P, ncols], I32, "idxt")
            iloct = load(rp, iloc, [P, ncols], BF, "iloct")
            rbftt = load(rp, rbft, [NRBF, s_slots], BF, "rbftt")
            envt = load(rp, envc, [P, ncols], FP, "envt")
            unitt = load(rp, unitc, [P, 3 * ncols], FP, "unitt")
            hloc = load(rp, hsh, [P, nt, F], FP, "hloc")
            vloc = rp.tile([P, nt, C], FP, tag="vloc")
            nc.vector.memset(vloc[:], 0.0)
            epst = cp.tile([P, 1], FP, tag="epst")
            nc.vector.memset(epst[:], EPS)
            zrow = rp.tile([P, C], BF, tag="zrow")
            nc.vector.memset(zrow[:], 0.0)

            # helper: PE transpose (in [P, m] bf16 -> sbuf [m, P] bf16)
            def transpose(in_ap, m, tag):
                pt = psE.tile([m, P], BF, tag="ept")
                nc.tensor.transpose(out=pt[:], in_=in_ap, identity=idn[:])
                st = ep.tile([m, P], BF, tag=tag)
                nc.scalar.copy(st[:], pt[:])
                return st

            # helper: node-tile phi MLP into pvx[l] (uses final H of the tile)
            def phi_into_pvx(l, t, rows, hfin_b16):
                hT = transpose(hfin_b16[:, :], F, f"hT{l}")
                pp1 = psE.tile([P, F], FP, tag="epm")
                nc.tensor.matmul(pp1[:], hT[:], w1[:, l, :], start=True, stop=False)
                nc.tensor.matmul(pp1[:], ones[:], b1[:, l, :], start=False, stop=True)
                a1 = wp.tile([P, F], BF, tag="phia1")
                nc.scalar.activation(a1[:], pp1[:], AF.Silu)
                a1T = transpose(a1[:, :], F, f"a1T{l}")
                pp2 = psE.tile([P, F4], FP, tag="epm")
                nc.tensor.matmul(pp2[:], a1T[:], w2[:, l, :], start=True, stop=False)
                nc.tensor.matmul(pp2[:], ones[:], b2[:, l, :], start=False, stop=True)
                px = wp.tile([P, F4], BF, tag="px")
                nc.scalar.copy(px[:], pp2[:])
                nc.sync.dma_start(out=pvx[l][t * P:t * P + rows, 0:F4],
                                  in_=px[:rows, :])

            # ---------------- prologue: phi_0 from input H, V_0 = 0
            for t in range(nt):
                rows = min(P, n_nodes // NCORE - t * P)
                hb = wp.tile([P, F], BF, tag="hb0")
                nc.vector.tensor_copy(hb[:], hloc[:, t, :])
                phi_into_pvx(0, t, rows, hb)
                nc.sync.dma_start(out=pvx[0][t * P:t * P + rows, F4:F4 + C],
                                  in_=zrow[:rows, :])

            # ---------------- layers
            for l in range(L):
                nc.gpsimd.collective_compute(
                    "AllGather", OP.bypass,
                    replica_groups=[list(range(NCORE))],
                    ins=[pvx[l][:, :]],
                    outs=[ptab[l][:, :]],
                )

                for t in range(nt):
                    rows = min(P, n_nodes // NCORE - t * P)
                    pA = psA.tile([P, F4], FP, tag="pA")
                    if l > 0:
                        pV = psA.tile([P, C], FP, tag="pV")
                        pW = psA.tile([P, C], FP, tag="pW")
                    for qq in range(q):
                        c = t * q + qq
                        g = gp.tile([P, F4 + C], BF, tag="g")
                        nc.gpsimd.indirect_dma_start(
                            out=g[:], out_offset=None,
                            in_=ptab[l][:, :],
                            in_offset=bass.IndirectOffsetOnAxis(
                                ap=idxt[:, c:c + 1], axis=0),
                        )
                        oh = gp.tile([P, P], BF, tag="oh")
                        nc.vector.tensor_tensor(
                            out=oh[:], in0=iloct[:, c:c + 1].to_broadcast([P, P]),
                            in1=iot[:], op=OP.is_equal)
                        pws = psW.tile([P, F4], FP, tag="ws")
                        nc.tensor.matmul(pws[:], rbftt[:, c * P:(c + 1) * P],
                                         rw[:, l, :], start=True, stop=False)
                        nc.tensor.matmul(pws[:], ones[:], rb[:, l, :],
                                         start=False, stop=True)
                        ws = gp.tile([P, F4], BF, tag="wss")
                        nc.scalar.activation(ws[:], pws[:], AF.Copy,
                                             scale=envt[:, c:c + 1])
                        R = gp.tile([P, 780], BF, tag="R")
                        # s0 = phi0 * ws0 -> R[0:78]
                        nc.vector.tensor_tensor(out=R[:, 0:F], in0=g[:, 0:F],
                                                in1=ws[:, 0:F], op=OP.mult)
                        iv = gp.tile([P, C], BF, tag="iv")  # [s1|s2|s3]
                        nc.vector.tensor_tensor(out=iv[:], in0=g[:, F:F4],
                                                in1=ws[:, F:F4], op=OP.mult)
                        # t1 = s2 (x) unit_c -> R[78:312]
                        for cc in range(3):
                            nc.scalar.activation(
                                R[:, F + cc * F:F + (cc + 1) * F], iv[:, F:2 * F],
                                AF.Copy, scale=unitt[:, 3 * c + cc:3 * c + cc + 1])
                        if l > 0:
                            for cc in range(3):
                                vj = g[:, F4 + cc * F:F4 + (cc + 1) * F]
                                nc.vector.tensor_tensor(
                                    out=R[:, F4 + cc * F:F4 + (cc + 1) * F],
                                    in0=iv[:, 0:F], in1=vj, op=OP.mult)
                                nc.vector.tensor_tensor(
                                    out=R[:, 546 + cc * F:546 + (cc + 1) * F],
                                    in0=iv[:, 2 * F:C], in1=vj, op=OP.mult)
                        nc.tensor.matmul(pA[:], oh[:], R[:, 0:F4],
                                         start=(qq == 0), stop=(qq == q - 1))
                        if l > 0:
                            nc.tensor.matmul(pV[:], oh[:], R[:, F4:546],
                                             start=(qq == 0), stop=(qq == q - 1))
                            nc.tensor.matmul(pW[:], oh[:], R[:, 546:780],
                                             start=(qq == 0), stop=(qq == q - 1))

                    # ---------------- tile epilogue
                    sA = ep.tile([P, F4], FP, tag="sA")
                    nc.scalar.copy(sA[:], pA[:])
                    if l > 0:
                        sV = ep.tile([P, C], FP, tag="sV")
                        nc.scalar.copy(sV[:], pV[:])
                        sW = ep.tile([P, C], FP, tag="sW")
                        nc.scalar.copy(sW[:], pW[:])

                    # H_mid = Hloc + s0-scatter
                    nc.vector.tensor_tensor(out=hloc[:, t, :], in0=hloc[:, t, :],
                                            in1=sA[:, 0:F], op=OP.add)
                    # V_mid = Vloc + t1 + t2 + cross(Vold, W)
                    vm = ep.tile([P, C], FP, tag="vm")
                    if l > 0:
                        cr = ep.tile([P, C], FP, tag="cr")
                        for cc in range(3):
                            c1, c2 = (cc + 1) % 3, (cc + 2) % 3
                            m1 = ep.tile([P, F], FP, tag="crm")
                            nc.vector.tensor_tensor(
                                out=m1[:], in0=vloc[:, t, c1 * F:(c1 + 1) * F],
                                in1=sW[:, c2 * F:(c2 + 1) * F], op=OP.mult)
                            m2 = ep.tile([P, F], FP, tag="crm2")
                            nc.vector.tensor_tensor(
                                out=m2[:], in0=vloc[:, t, c2 * F:(c2 + 1) * F],
                                in1=sW[:, c1 * F:(c1 + 1) * F], op=OP.mult)
                            nc.vector.tensor_tensor(
                                out=cr[:, cc * F:(cc + 1) * F], in0=m1[:],
                                in1=m2[:], op=OP.subtract)
                        nc.vector.tensor_tensor(out=cr[:], in0=cr[:], in1=sV[:],
                                                op=OP.add)
                        nc.vector.tensor_tensor(out=cr[:], in0=cr[:],
                                                in1=sA[:, F:F4], op=OP.add)
                        nc.vector.tensor_tensor(out=vm[:], in0=vloc[:, t, :],
                                                in1=cr[:], op=OP.add)
                    else:
                        nc.vector.tensor_tensor(out=vm[:], in0=vloc[:, t, :],
                                                in1=sA[:, F:F4], op=OP.add)
                    nc.vector.tensor_copy(vloc[:, t, :], vm[:])

                    # ---- update block
                    vmb = ep.tile([P, C], BF, tag="vmb")
                    nc.vector.tensor_copy(vmb[:], vm[:])
                    puv = psE.tile([P, C], FP, tag="epm")
                    pvv = psE.tile([P, C], FP, tag="epm")
                    for cc in range(3):
                        vT = transpose(vmb[:, cc * F:(cc + 1) * F], F, "vT")
                        nc.tensor.matmul(puv[:, cc * F:(cc + 1) * F], vT[:],
                                         wu[:, l, :], start=True, stop=True)
                        nc.tensor.matmul(pvv[:, cc * F:(cc + 1) * F], vT[:],
                                         wv[:, l, :], start=True, stop=True)
                    uv = ep.tile([P, C], FP, tag="uv")
                    nc.scalar.copy(uv[:], puv[:])
                    vv = ep.tile([P, C], FP, tag="vv")
                    nc.scalar.copy(vv[:], pvv[:])
                    # v_norm
                    v2 = ep.tile([P, F], FP, tag="v2")
                    nc.vector.tensor_tensor(out=v2[:], in0=vv[:, 0:F],
                                            in1=vv[:, 0:F], op=OP.mult)
                    for cc in (1, 2):
                        m = ep.tile([P, F], FP, tag="v2m")
                        nc.vector.tensor_tensor(
                            out=m[:], in0=vv[:, cc * F:(cc + 1) * F],
                            in1=vv[:, cc * F:(cc + 1) * F], op=OP.mult)
                        nc.vector.tensor_tensor(out=v2[:], in0=v2[:], in1=m[:],
                                                op=OP.add)
                    vn = ep.tile([P, F], BF, tag="vn")
                    nc.scalar.activation(vn[:], v2[:], AF.Sqrt, bias=epst[:, :])
                    vnT = transpose(vn[:, :], F, "vnT")
                    hmb = ep.tile([P, F], BF, tag="hmb")
                    nc.vector.tensor_copy(hmb[:], hloc[:, t, :])
                    hmT = transpose(hmb[:, :], F, "hmT")
                    pa1 = psE.tile([P, F], FP, tag="epm")
                    nc.tensor.matmul(pa1[:], hmT[:], w1a[:, l, :], start=True, stop=False)
                    nc.tensor.matmul(pa1[:], vnT[:], w1b[:, l, :], start=False, stop=False)
                    nc.tensor.matmul(pa1[:], ones[:], b1u[:, l, :], start=False, stop=True)
                    a1 = ep.tile([P, F], BF, tag="ua1")
                    nc.scalar.activation(a1[:], pa1[:], AF.Silu)
                    a1T = transpose(a1[:, :], F, "ua1T")
                    pa2 = psE.tile([P, 3 * F], FP, tag="epm")
                    nc.tensor.matmul(pa2[:], a1T[:], w2u[:, l, :], start=True, stop=False)
                    nc.tensor.matmul(pa2[:], ones[:], b2u[:, l, :], start=False, stop=True)
                    # H += a_sv * (uv . vv) + a_ss
                    dot = ep.tile([P, F], FP, tag="dot")
                    nc.vector.tensor_tensor(out=dot[:], in0=uv[:, 0:F],
                                            in1=vv[:, 0:F], op=OP.mult)
                    for cc in (1, 2):
                        m = ep.tile([P, F], FP, tag="dotm")
                        nc.vector.tensor_tensor(
                            out=m[:], in0=uv[:, cc * F:(cc + 1) * F],
                            in1=vv[:, cc * F:(cc + 1) * F], op=OP.mult)
                        nc.vector.tensor_tensor(out=dot[:], in0=dot[:], in1=m[:],
                                                op=OP.add)
                    hadd = ep.tile([P, F], FP, tag="hadd")
                    nc.vector.tensor_tensor(out=hadd[:], in0=pa2[:, F:2 * F],
                                            in1=dot[:], op=OP.mult)
                    nc.vector.tensor_tensor(out=hadd[:], in0=hadd[:],
                                            in1=pa2[:, 2 * F:3 * F], op=OP.add)
                    nc.vector.tensor_tensor(out=hloc[:, t, :], in0=hloc[:, t, :],
                                            in1=hadd[:], op=OP.add)
                    # V += a_vv (x) u_v
                    avv = ep.tile([P, F], FP, tag="avv")
                    nc.scalar.copy(avv[:], pa2[:, 0:F])
                    for cc in range(3):
                        m = ep.tile([P, F], FP, tag="vadd")
                        nc.vector.tensor_tensor(
                            out=m[:], in0=avv[:], in1=uv[:, cc * F:(cc + 1) * F],
                            op=OP.mult)
                        nc.vector.tensor_tensor(
                            out=vloc[:, t, cc * F:(cc + 1) * F],
                            in0=vloc[:, t, cc * F:(cc + 1) * F], in1=m[:],
                            op=OP.add)

                    if l < L - 1:
                        # stage phi_{l+1} | V_{l+1} rows for the next exchange
                        hfb = ep.tile([P, F], BF, tag="hfb")
                        nc.vector.tensor_copy(hfb[:], hloc[:, t, :])
                        phi_into_pvx(l + 1, t, rows, hfb)
                        vfb = ep.tile([P, C], BF, tag="vfb")
                        nc.vector.tensor_copy(vfb[:], vloc[:, t, :])
                        nc.sync.dma_start(
                            out=pvx[l + 1][t * P:t * P + rows, F4:F4 + C],
                            in_=vfb[:rows, :])
                    else:
                        # final head
                        vs = ep.tile([P, F], FP, tag="vs")
                        nc.vector.tensor_tensor(out=vs[:], in0=vloc[:, t, 0:F],
                                                in1=vloc[:, t, F:2 * F], op=OP.add)
                        nc.vector.tensor_tensor(out=vs[:], in0=vs[:],
                                                in1=vloc[:, t, 2 * F:C], op=OP.add)
                        vsb = ep.tile([P, F], BF, tag="vsb")
                        nc.scalar.activation(vsb[:], vs[:], AF.Relu)
                        vsT = transpose(vsb[:, :], F, "vsT")
                        ph1 = psE.tile([P, 39], FP, tag="epm")
                        nc.tensor.matmul(ph1[:], vsT[:], hw1[:, :], start=True, stop=False)
                        nc.tensor.matmul(ph1[:], ones[:], hb1[:, :], start=False, stop=True)
                        h1 = ep.tile([P, 39], BF, tag="h1")
                        nc.scalar.activation(h1[:], ph1[:], AF.Relu)
                        h1T = transpose(h1[:, :], 39, "h1T")
                        ph2 = psE.tile([P, 39], FP, tag="epm")
                        nc.tensor.matmul(ph2[:], h1T[:], hw2[:, :], start=True, stop=False)
                        nc.tensor.matmul(ph2[:], ones[:], hb2[:, :], start=False, stop=True)
                        yo = ep.tile([P, 39], FP, tag="yo")
                        nc.scalar.copy(yo[:], ph2[:])
                        nc.sync.dma_start(out=y_out[t * P:t * P + rows, :],
                                          in_=yo[:rows, :])
                        nc.sync.dma_start(out=h_out[t * P:t * P + rows, :],
                                          in_=hloc[:rows, t, :])

    return nc


# ---------------------------------------------------------------- host prep
_CACHE = {}


def _get_nc(n_nodes, nt, q):
    key = (n_nodes, nt, q)
    if key not in _CACHE:
        _apply_patches()
        nc = _build_nc(n_nodes, nt, q)
        _split_multiwaits(nc)
        _CACHE[key] = nc
    return _CACHE[key]


def kernel(cg_xyz, CG_nbr_list, mapping, H,
           msg_w1, msg_b1, msg_w2, msg_b2, rbf_w, rbf_b,
           upd_wu, upd_wv, upd_w1, upd_b1, upd_w2, upd_b2,
           dense_w1, dense_b1, dense_w2, dense_b2):
    xyz = np.asarray(cg_xyz, np.float32)
    nbr = np.asarray(CG_nbr_list)
    H0 = np.asarray(H, np.float32)
    n_nodes = H0.shape[0]
    nsh = n_nodes // NCORE
    nt = (nsh + P - 1) // P

    ii = np.asarray(nbr[:, 0], np.int64)
    jj = np.asarray(nbr[:, 1], np.int64)
    r = xyz[jj] - xyz[ii]
    dist = np.sqrt((r.astype(np.float32) ** 2 + EPS).sum(-1))
    keep = dist < CUTOFF
    ii, jj, r, dist = ii[keep], jj[keep], r[keep], dist[keep]
    env = 0.5 * (np.cos(np.pi * dist / CUTOFF) + 1.0)
    unit = r / dist[:, None]
    nvec = np.arange(1, NRBF + 1, dtype=np.float32)
    rbf = np.sin(nvec[None, :] * np.pi * dist[:, None] / CUTOFF) / dist[:, None]

    core = ii // nsh
    # per-core, per-tile edge counts -> global q
    q = 1
    percore = []
    for k in range(NCORE):
        m = core == k
        il = (ii[m] - k * nsh).astype(np.int64)
        order = np.argsort(il, kind="stable")
        dat = dict(il=il[order], j=jj[m][order], env=env[m][order],
                   unit=unit[m][order], rbf=rbf[m][order])
        tiles = dat["il"] // P
        cnt = np.bincount(tiles, minlength=nt)
        q = max(q, int(np.ceil(cnt.max() / P)) if cnt.max() else 1)
        percore.append((dat, cnt))

    ncols = nt * q
    s_slots = ncols * P
    bf16 = ml_dtypes.bfloat16

    def wslice(w):  # [L, A, B] -> [A, L, B]
        return np.ascontiguousarray(np.transpose(np.asarray(w, np.float32),
                                                 (1, 0, 2))).astype(bf16)

    wcom = {
        "w1h": wslice(msg_w1), "b1h": wslice(np.asarray(msg_b1)[:, None, :]),
        "w2h": wslice(msg_w2), "b2h": wslice(np.asarray(msg_b2)[:, None, :]),
        "rwh": wslice(rbf_w), "rbh": wslice(np.asarray(rbf_b)[:, None, :]),
        "wuh": wslice(upd_wu), "wvh": wslice(upd_wv),
        "u1a": wslice(np.asarray(upd_w1)[:, 0:F, :]),
        "u1b": wslice(np.asarray(upd_w1)[:, F:2 * F, :]),
        "ub1": wslice(np.asarray(upd_b1)[:, None, :]),
        "u2h": wslice(upd_w2), "ub2": wslice(np.asarray(upd_b2)[:, None, :]),
        "dw1": np.asarray(dense_w1, np.float32).astype(bf16),
        "db1": np.asarray(dense_b1, np.float32)[None, :].astype(bf16),
        "dw2": np.asarray(dense_w2, np.float32).astype(bf16),
        "db2": np.asarray(dense_b2, np.float32)[None, :].astype(bf16),
        "onesr": np.ones((1, P), bf16),
        "iota": np.tile(np.arange(P, dtype=np.float32), (P, 1)).astype(bf16),
        "ident": np.eye(P, dtype=np.float32).astype(bf16),
    }

    in_maps = []
    for k in range(NCORE):
        dat, cnt = percore[k]
        idx_s = np.zeros(s_slots, np.int32)
        iloc_s = np.full(s_slots, 200.0, np.float32)
        env_s = np.zeros(s_slots, np.float32)
        unit_s = np.zeros((s_slots, 3), np.float32)
        rbf_s = np.zeros((s_slots, NRBF), np.float32)
        pos = 0
        for t in range(nt):
            n_e = int(cnt[t])
            sl = slice(t * q * P, t * q * P + n_e)
            idx_s[sl] = dat["j"][pos:pos + n_e]
            iloc_s[sl] = dat["il"][pos:pos + n_e] - t * P
            env_s[sl] = dat["env"][pos:pos + n_e]
            unit_s[sl] = dat["unit"][pos:pos + n_e]
            rbf_s[sl] = dat["rbf"][pos:pos + n_e]
            pos += n_e

        hshard = np.zeros((P, nt, F), np.float32)
        hv = H0[k * nsh:(k + 1) * nsh]
        hpad = np.zeros((nt * P, F), np.float32)
        hpad[:nsh] = hv
        hshard[:, :, :] = hpad.reshape(nt, P, F).transpose(1, 0, 2)

        m = dict(wcom)
        m["idxj"] = np.ascontiguousarray(idx_s.reshape(ncols, P).T)
        m["iloc"] = np.ascontiguousarray(iloc_s.reshape(ncols, P).T).astype(bf16)
        m["rbft"] = np.ascontiguousarray(rbf_s.T).astype(bf16)
        m["envc"] = np.ascontiguousarray(env_s.reshape(ncols, P).T)
        m["unitc"] = np.ascontiguousarray(
            unit_s.reshape(ncols, P, 3).transpose(1, 0, 2).reshape(P, ncols * 3))
        m["hsh"] = hshard
        in_maps.append(m)

    nc = _get_nc(n_nodes, nt, q)
    res = run_bass_kernel_spmd(nc, in_maps, core_ids=list(range(NCORE)))

    h_full = np.concatenate([res.results[k]["h_out"][:nsh] for k in range(NCORE)], 0)
    y_full = np.concatenate([res.results[k]["y_out"][:nsh] for k in range(NCORE)], 0)
    return h_full.astype(np.float32), y_full.reshape(-1, 13, 3).astype(np.float32)


# revision 12
# speedup vs baseline: 1.1970x; 1.1970x over previous
"""Trainium2 Bass kernel for nn_EquivariantDecoder (PaiNN-style equivariant GNN).

Strategy (8 NeuronCores, graph-parallel):
  - Host: drop edges with env==0 (dist >= cutoff, exactly zero contribution),
    shard nodes across cores, degree-balance nodes into destination tiles,
    sort each core's edges by destination tile.
  - Device, per layer: AllGather of per-node [phi | V] bf16 rows builds a
    global gather table in each core's HBM; per 128-edge chunk an indirect
    DMA gathers source rows; edge messages are elementwise DVE/ACT work
    batched per tile; segment sums are one-hot matmuls accumulating in PSUM
    (destination-sorted edges keep scatters core-local). The cross-product
    term uses linearity, cross(V[i], sum s3*V[j]) evaluated per node. The
    cutoff envelope rides as a 21st RBF feature so w_s is a single matmul.
    Update block + next layer's phi MLP run node-locally; a final dense head
    produces the outputs.
"""
import numpy as np
import ml_dtypes

import concourse.bass as bass
import concourse.tile as tile
from concourse import mybir
from concourse.bass_utils import run_bass_kernel_spmd
from concourse.vector_clock import ScopedClock

# ---------------------------------------------------------------- constants
F = 78
F4 = 4 * F          # 312
C = 3 * F           # 234
NRBF = 20
NRB1 = NRBF + 1     # + envelope feature
CUTOFF = 21.0
L = 3
EPS = 1e-15
NCORE = 8
P = 128
BF = mybir.dt.bfloat16
FP = mybir.dt.float32
I32 = mybir.dt.int32
AF = mybir.ActivationFunctionType
TRACE_SIM = False
OP = mybir.AluOpType

# ------------------------------------------------------- walrus workarounds
_PATCHED = False


def _apply_patches():
    """This walrus build accepts only ONE sync-wait per instruction.
    (a) patch TileContext's exit drain to spread its final-clock waits over
    single-wait SP nops; (b) post-pass splitting any instruction's extra
    waits onto same-engine nop carriers."""
    global _PATCHED
    if _PATCHED:
        return
    _PATCHED = True

    def _drain_and_barrier(self, tick_clock, wait_clock):
        nc = self.nc
        carrier = nc.sync.nop(nofuse=True, hint="drain_waits").ins
        wait_clock.add_sem_waits(carrier, ScopedClock({None: tick_clock.global_clock}))
        waits = list(carrier.sync_info.on_wait or [])
        if len(waits) > 1:
            carrier.sync_info.on_wait = waits[:1]
            for w in waits[1:]:
                c2 = nc.sync.nop(nofuse=True, hint="drain_waits").ins
                c2.sync_info = mybir.SyncInfo(on_wait=[w], on_update=[])
        nc.sync.drain()
        nc.all_engine_barrier()
        popped = nc._tile_sem_poison_stack.pop()
        assert popped is self._sem_poison
        nc.clear_and_free_semaphores(list(self.sems.allocated().values()))
        nc.all_engine_barrier()

    tile.TileContext._drain_and_barrier = _drain_and_barrier


_WS_CNT = [0]


def _split_multiwaits(nc):
    for f in nc.m.functions:
        for bb in f.blocks:
            out = []
            changed = False
            for inst in bb.instructions:
                si = getattr(inst, "sync_info", None)
                if si is not None and si.on_wait and len(si.on_wait) > 1:
                    waits = list(si.on_wait)
                    for w in waits[:-1]:
                        _WS_CNT[0] += 1
                        out.append(mybir.InstNoOp(
                            name=f"WS-{_WS_CNT[0]}-{inst.name}",
                            engine=inst.engine,
                            bass_nofuse=True,
                            sync_info=mybir.SyncInfo(on_wait=[w], on_update=[]),
                        ))
                    si.on_wait = waits[-1:]
                    changed = True
                out.append(inst)
            if changed:
                try:
                    bb.instructions[:] = out
                except TypeError:
                    bb.instructions = out


# -------------------------------------------------------------- device build
def _build_nc(n_nodes, nt, q):
    """One SPMD program for all 8 cores. nt destination tiles of ntr nodes per
    core, q chunks of 128 edges per tile."""
    nsh = n_nodes // NCORE
    ntr = nsh // nt                 # nodes per tile (<= 128)
    ncols = nt * q
    s_slots = ncols * P
    trow = 552                      # ptab row: [phi 312 | V 234 | pad 6] bf16

    nc = bass.Bass()

    def din(name, shape, dt):
        return nc.dram_tensor(name, shape, dt, kind="ExternalInput")

    idxj = din("idxj", [P, ncols], I32)
    iloc = din("iloc", [P, ncols], BF)
    rbft = din("rbft", [NRB1, s_slots], BF)
    unitc = din("unitc", [P, 3 * ncols], FP)
    hsh = din("hsh", [P, nt, F], FP)
    w1h = din("w1h", [F, L, F], BF);    b1h = din("b1h", [1, L, F], BF)
    w2h = din("w2h", [F, L, F4], BF);   b2h = din("b2h", [1, L, F4], BF)
    rwh = din("rwh", [NRB1, L, F4], BF)
    wuh = din("wuh", [F, L, F], BF);    wvh = din("wvh", [F, L, F], BF)
    u1a = din("u1a", [F, L, F], BF);    u1b = din("u1b", [F, L, F], BF)
    ub1 = din("ub1", [1, L, F], BF)
    u2h = din("u2h", [F, L, 3 * F], BF); ub2 = din("ub2", [1, L, 3 * F], BF)
    dw1 = din("dw1", [F, 39], BF);      db1 = din("db1", [1, 39], BF)
    dw2 = din("dw2", [39, 39], BF);     db2 = din("db2", [1, 39], BF)
    onesr = din("onesr", [1, P], BF)
    iota = din("iota", [P, P], BF)
    ident = din("ident", [P, P], BF)

    h_out = nc.dram_tensor("h_out", [nsh, F], FP, kind="ExternalOutput")
    y_out = nc.dram_tensor("y_out", [nsh, 39], FP, kind="ExternalOutput")

    pvx = [nc.dram_tensor(f"pvx{l}", [nsh, trow], BF) for l in range(L)]
    ptab = [nc.dram_tensor(f"ptab{l}", [NCORE * nsh, trow], BF,
                           addr_space="Shared") for l in range(L)]

    with tile.TileContext(nc, trace_sim=TRACE_SIM) as tc:
        with (
            tc.tile_pool(name="const", bufs=1) as cp,
            tc.tile_pool(name="res", bufs=1) as rp,
            tc.tile_pool(name="work", bufs=3) as wp,
            tc.tile_pool(name="gat", bufs=2) as gp,
            tc.tile_pool(name="ep", bufs=2) as ep,
            tc.tile_pool(name="psA", bufs=1, space="PSUM") as psA,
            tc.tile_pool(name="psW", bufs=1, space="PSUM") as psW,
            tc.tile_pool(name="psE", bufs=2, space="PSUM") as psE,
        ):
            def load(pool, src, shape, dt, tag):
                t = pool.tile(shape, dt, tag=tag)
                nc.sync.dma_start(out=t[:], in_=src[:])
                return t

            w1 = load(cp, w1h, [F, L, F], BF, "w1");   b1 = load(cp, b1h, [1, L, F], BF, "b1")
            w2 = load(cp, w2h, [F, L, F4], BF, "w2");  b2 = load(cp, b2h, [1, L, F4], BF, "b2")
            rw = load(cp, rwh, [NRB1, L, F4], BF, "rw")
            wu = load(cp, wuh, [F, L, F], BF, "wu");   wv = load(cp, wvh, [F, L, F], BF, "wv")
            w1a = load(cp, u1a, [F, L, F], BF, "w1a"); w1b = load(cp, u1b, [F, L, F], BF, "w1b")
            b1u = load(cp, ub1, [1, L, F], BF, "b1u")
            w2u = load(cp, u2h, [F, L, 3 * F], BF, "w2u")
            b2u = load(cp, ub2, [1, L, 3 * F], BF, "b2u")
            hw1 = load(cp, dw1, [F, 39], BF, "hw1");   hb1 = load(cp, db1, [1, 39], BF, "hb1")
            hw2 = load(cp, dw2, [39, 39], BF, "hw2");  hb2 = load(cp, db2, [1, 39], BF, "hb2")
            ones = load(cp, onesr, [1, P], BF, "ones")
            iot = load(cp, iota, [P, P], BF, "iot")
            idn = load(cp, ident, [P, P], BF, "idn")
            epst = cp.tile([P, 1], FP, tag="epst")
            nc.vector.memset(epst[:], EPS)

            idxt = load(rp, idxj, [P, ncols], I32, "idxt")
            iloct = load(rp, iloc, [P, ncols], BF, "iloct")
            rbftt = load(rp, rbft, [NRB1, s_slots], BF, "rbftt")
            unitt = load(rp, unitc, [P, 3 * ncols], FP, "unitt")
            hloc = load(rp, hsh, [P, nt, F], FP, "hloc")
            vloc = rp.tile([P, nt, C], FP, tag="vloc")
            nc.vector.memset(vloc[:], 0.0)
            zrow = rp.tile([P, C], BF, tag="zrow")
            nc.vector.memset(zrow[:], 0.0)

            def transpose(in_ap, m, tag):
                pt = psE.tile([m, P], BF, tag="ept")
                nc.tensor.transpose(out=pt[:], in_=in_ap, identity=idn[:])
                st = ep.tile([m, P], BF, tag=tag)
                nc.scalar.copy(st[:], pt[:])
                return st

            def phi_into_pvx(l, t, hfin_b16):
                hT = transpose(hfin_b16[:, :], F, "hT")
                pp1 = psE.tile([P, F], FP, tag="epm")
                nc.tensor.matmul(pp1[:], hT[:], w1[:, l, :], start=True, stop=False)
                nc.tensor.matmul(pp1[:], ones[:], b1[:, l, :], start=False, stop=True)
                a1 = wp.tile([P, F], BF, tag="phia1")
                nc.scalar.activation(a1[:], pp1[:], AF.Silu)
                a1T = transpose(a1[:, :], F, "a1T")
                pp2 = psE.tile([P, F4], FP, tag="epm")
                nc.tensor.matmul(pp2[:], a1T[:], w2[:, l, :], start=True, stop=False)
                nc.tensor.matmul(pp2[:], ones[:], b2[:, l, :], start=False, stop=True)
                px = wp.tile([P, F4], BF, tag="px")
                nc.scalar.copy(px[:], pp2[:])
                nc.sync.dma_start(out=pvx[l][t * ntr:(t + 1) * ntr, 0:F4],
                                  in_=px[:ntr, :])

            # ---------------- prologue: phi_0 from input H, V_0 = 0
            for t in range(nt):
                hb = wp.tile([P, F], BF, tag="hb0")
                nc.vector.tensor_copy(hb[:], hloc[:, t, :])
                phi_into_pvx(0, t, hb)
                nc.sync.dma_start(out=pvx[0][t * ntr:(t + 1) * ntr, F4:F4 + C],
                                  in_=zrow[:ntr, :])

            # ---------------- layers
            for l in range(L):
                nc.gpsimd.collective_compute(
                    "AllGather", OP.bypass,
                    replica_groups=[list(range(NCORE))],
                    ins=[pvx[l][:, :]],
                    outs=[ptab[l][:, :]],
                )

                for t in range(nt):
                    c0 = t * q
                    pA = psA.tile([P, F4], FP, tag="pA")
                    if l > 0:
                        pV = psA.tile([P, C], FP, tag="pV")
                        pW = psA.tile([P, C], FP, tag="pW")
                    g9 = gp.tile([P, q, F4 + C], BF, tag="g")
                    ws9 = gp.tile([P, q, F4], BF, tag="wss")
                    for qq in range(q):
                        c = c0 + qq
                        nc.gpsimd.indirect_dma_start(
                            out=g9[:, qq, :], out_offset=None,
                            in_=ptab[l][:, :],
                            in_offset=bass.IndirectOffsetOnAxis(
                                ap=idxt[:, c:c + 1], axis=0),
                        )
                        pws = psW.tile([P, F4], FP, tag="ws")
                        nc.tensor.matmul(pws[:], rbftt[:, c * P:(c + 1) * P],
                                         rw[:, l, :], start=True, stop=True)
                        nc.scalar.copy(ws9[:, qq, :], pws[:])
                    oh9 = gp.tile([P, q, P], BF, tag="oh")
                    nc.vector.tensor_tensor(
                        out=oh9[:], in0=iloct[:, c0:c0 + q].to_broadcast([P, q, P]),
                        in1=iot[:, None, :].to_broadcast([P, q, P]), op=OP.is_equal)
                    R9 = gp.tile([P, q, 780], BF, tag="R")
                    nc.vector.tensor_tensor(out=R9[:, :, 0:F], in0=g9[:, :, 0:F],
                                            in1=ws9[:, :, 0:F], op=OP.mult)
                    iv9 = gp.tile([P, q, C], BF, tag="iv")
                    nc.vector.tensor_tensor(out=iv9[:], in0=g9[:, :, F:F4],
                                            in1=ws9[:, :, F:F4], op=OP.mult)
                    for qq in range(q):
                        c = c0 + qq
                        for cc in range(3):
                            nc.vector.tensor_scalar(
                                out=R9[:, qq, F + cc * F:F + (cc + 1) * F],
                                in0=iv9[:, qq, F:2 * F],
                                scalar1=unitt[:, 3 * c + cc:3 * c + cc + 1],
                                scalar2=None, op0=OP.mult)
                    if l > 0:
                        vj = g9[:, :, F4:F4 + C].rearrange("p q (c f) -> p q c f", c=3)
                        nc.vector.tensor_tensor(
                            out=R9[:, :, F4:546].rearrange("p q (c f) -> p q c f", c=3),
                            in0=iv9[:, :, None, 0:F].to_broadcast([P, q, 3, F]),
                            in1=vj, op=OP.mult)
                        nc.vector.tensor_tensor(
                            out=R9[:, :, 546:780].rearrange("p q (c f) -> p q c f", c=3),
                            in0=iv9[:, :, None, 2 * F:C].to_broadcast([P, q, 3, F]),
                            in1=vj, op=OP.mult)
                    for qq in range(q):
                        nc.tensor.matmul(pA[:], oh9[:, qq, :], R9[:, qq, 0:F4],
                                         start=(qq == 0), stop=(qq == q - 1))
                        if l > 0:
                            nc.tensor.matmul(pV[:], oh9[:, qq, :], R9[:, qq, F4:546],
                                             start=(qq == 0), stop=(qq == q - 1))
                            nc.tensor.matmul(pW[:], oh9[:, qq, :], R9[:, qq, 546:780],
                                             start=(qq == 0), stop=(qq == q - 1))

                    # ---------------- tile epilogue
                    sA = ep.tile([P, F4], FP, tag="sA")
                    nc.scalar.copy(sA[:], pA[:])
                    if l > 0:
                        sV = ep.tile([P, C], FP, tag="sV")
                        nc.scalar.copy(sV[:], pV[:])
                        sW = ep.tile([P, C], FP, tag="sW")
                        nc.scalar.copy(sW[:], pW[:])

                    nc.vector.tensor_tensor(out=hloc[:, t, :], in0=hloc[:, t, :],
                                            in1=sA[:, 0:F], op=OP.add)
                    vm = ep.tile([P, C], FP, tag="vm")
                    if l > 0:
                        cr = ep.tile([P, C], FP, tag="cr")
                        for cc in range(3):
                            c1, c2 = (cc + 1) % 3, (cc + 2) % 3
                            m1 = ep.tile([P, F], FP, tag="crm")
                            nc.vector.tensor_tensor(
                                out=m1[:], in0=vloc[:, t, c1 * F:(c1 + 1) * F],
                                in1=sW[:, c2 * F:(c2 + 1) * F], op=OP.mult)
                            m2 = ep.tile([P, F], FP, tag="crm2")
                            nc.vector.tensor_tensor(
                                out=m2[:], in0=vloc[:, t, c2 * F:(c2 + 1) * F],
                                in1=sW[:, c1 * F:(c1 + 1) * F], op=OP.mult)
                            nc.vector.tensor_tensor(
                                out=cr[:, cc * F:(cc + 1) * F], in0=m1[:],
                                in1=m2[:], op=OP.subtract)
                        nc.vector.tensor_tensor(out=cr[:], in0=cr[:], in1=sV[:],
                                                op=OP.add)
                        nc.vector.tensor_tensor(out=cr[:], in0=cr[:],
                                                in1=sA[:, F:F4], op=OP.add)
                        nc.vector.tensor_tensor(out=vm[:], in0=vloc[:, t, :],
                                                in1=cr[:], op=OP.add)
                    else:
                        nc.vector.tensor_tensor(out=vm[:], in0=vloc[:, t, :],
                                                in1=sA[:, F:F4], op=OP.add)
                    nc.vector.tensor_copy(vloc[:, t, :], vm[:])

                    # ---- update block
                    vmb = ep.tile([P, C], BF, tag="vmb")
                    nc.vector.tensor_copy(vmb[:], vm[:])
                    puv = psE.tile([P, C], FP, tag="epm")
                    pvv = psE.tile([P, C], FP, tag="epm")
                    for cc in range(3):
                        vT = transpose(vmb[:, cc * F:(cc + 1) * F], F, "vT")
                        nc.tensor.matmul(puv[:, cc * F:(cc + 1) * F], vT[:],
                                         wu[:, l, :], start=True, stop=True)
                        nc.tensor.matmul(pvv[:, cc * F:(cc + 1) * F], vT[:],
                                         wv[:, l, :], start=True, stop=True)
                    uv = ep.tile([P, C], FP, tag="uv")
                    nc.scalar.copy(uv[:], puv[:])
                    vv = ep.tile([P, C], FP, tag="vv")
                    nc.scalar.copy(vv[:], pvv[:])
                    v2 = ep.tile([P, F], FP, tag="v2")
                    nc.vector.tensor_tensor(out=v2[:], in0=vv[:, 0:F],
                                            in1=vv[:, 0:F], op=OP.mult)
                    for cc in (1, 2):
                        m = ep.tile([P, F], FP, tag="v2m")
                        nc.vector.tensor_tensor(
                            out=m[:], in0=vv[:, cc * F:(cc + 1) * F],
                            in1=vv[:, cc * F:(cc + 1) * F], op=OP.mult)
                        nc.vector.tensor_tensor(out=v2[:], in0=v2[:], in1=m[:],
                                                op=OP.add)
                    vn = ep.tile([P, F], BF, tag="vn")
                    nc.scalar.activation(vn[:], v2[:], AF.Sqrt, bias=epst[:, :])
                    vnT = transpose(vn[:, :], F, "vnT")
                    hmb = ep.tile([P, F], BF, tag="hmb")
                    nc.vector.tensor_copy(hmb[:], hloc[:, t, :])
                    hmT = transpose(hmb[:, :], F, "hmT")
                    pa1 = psE.tile([P, F], FP, tag="epm")
                    nc.tensor.matmul(pa1[:], hmT[:], w1a[:, l, :], start=True, stop=False)
                    nc.tensor.matmul(pa1[:], vnT[:], w1b[:, l, :], start=False, stop=False)
                    nc.tensor.matmul(pa1[:], ones[:], b1u[:, l, :], start=False, stop=True)
                    a1 = ep.tile([P, F], BF, tag="ua1")
                    nc.scalar.activation(a1[:], pa1[:], AF.Silu)
                    a1T = transpose(a1[:, :], F, "ua1T")
                    pa2 = psE.tile([P, 3 * F], FP, tag="epm")
                    nc.tensor.matmul(pa2[:], a1T[:], w2u[:, l, :], start=True, stop=False)
                    nc.tensor.matmul(pa2[:], ones[:], b2u[:, l, :], start=False, stop=True)
                    dot = ep.tile([P, F], FP, tag="dot")
                    nc.vector.tensor_tensor(out=dot[:], in0=uv[:, 0:F],
                                            in1=vv[:, 0:F], op=OP.mult)
                    for cc in (1, 2):
                        m = ep.tile([P, F], FP, tag="dotm")
                        nc.vector.tensor_tensor(
                            out=m[:], in0=uv[:, cc * F:(cc + 1) * F],
                            in1=vv[:, cc * F:(cc + 1) * F], op=OP.mult)
                        nc.vector.tensor_tensor(out=dot[:], in0=dot[:], in1=m[:],
                                                op=OP.add)
                    hadd = ep.tile([P, F], FP, tag="hadd")
                    nc.vector.tensor_tensor(out=hadd[:], in0=pa2[:, F:2 * F],
                                            in1=dot[:], op=OP.mult)
                    nc.vector.tensor_tensor(out=hadd[:], in0=hadd[:],
                                            in1=pa2[:, 2 * F:3 * F], op=OP.add)
                    nc.vector.tensor_tensor(out=hloc[:, t, :], in0=hloc[:, t, :],
                                            in1=hadd[:], op=OP.add)
                    avv = ep.tile([P, F], FP, tag="avv")
                    nc.scalar.copy(avv[:], pa2[:, 0:F])
                    for cc in range(3):
                        m = ep.tile([P, F], FP, tag="vadd")
                        nc.vector.tensor_tensor(
                            out=m[:], in0=avv[:], in1=uv[:, cc * F:(cc + 1) * F],
                            op=OP.mult)
                        nc.vector.tensor_tensor(
                            out=vloc[:, t, cc * F:(cc + 1) * F],
                            in0=vloc[:, t, cc * F:(cc + 1) * F], in1=m[:],
                            op=OP.add)

                    if l < L - 1:
                        hfb = ep.tile([P, F], BF, tag="hfb")
                        nc.vector.tensor_copy(hfb[:], hloc[:, t, :])
                        phi_into_pvx(l + 1, t, hfb)
                        vfb = ep.tile([P, C], BF, tag="vfb")
                        nc.vector.tensor_copy(vfb[:], vloc[:, t, :])
                        nc.sync.dma_start(
                            out=pvx[l + 1][t * ntr:(t + 1) * ntr, F4:F4 + C],
                            in_=vfb[:ntr, :])
                    else:
                        vs = ep.tile([P, F], FP, tag="vs")
                        nc.vector.tensor_tensor(out=vs[:], in0=vloc[:, t, 0:F],
                                                in1=vloc[:, t, F:2 * F], op=OP.add)
                        nc.vector.tensor_tensor(out=vs[:], in0=vs[:],
                                                in1=vloc[:, t, 2 * F:C], op=OP.add)
                        vsb = ep.tile([P, F], BF, tag="vsb")
                        nc.scalar.activation(vsb[:], vs[:], AF.Relu)
                        vsT = transpose(vsb[:, :], F, "vsT")
                        ph1 = psE.tile([P, 39], FP, tag="epm")
                        nc.tensor.matmul(ph1[:], vsT[:], hw1[:, :], start=True, stop=False)
                        nc.tensor.matmul(ph1[:], ones[:], hb1[:, :], start=False, stop=True)
                        h1 = ep.tile([P, 39], BF, tag="h1")
                        nc.scalar.activation(h1[:], ph1[:], AF.Relu)
                        h1T = transpose(h1[:, :], 39, "h1T")
                        ph2 = psE.tile([P, 39], FP, tag="epm")
                        nc.tensor.matmul(ph2[:], h1T[:], hw2[:, :], start=True, stop=False)
                        nc.tensor.matmul(ph2[:], ones[:], hb2[:, :], start=False, stop=True)
                        yo = ep.tile([P, 39], FP, tag="yo")
                        nc.scalar.copy(yo[:], ph2[:])
                        nc.sync.dma_start(out=y_out[t * ntr:(t + 1) * ntr, :],
                                          in_=yo[:ntr, :])
                        nc.sync.dma_start(out=h_out[t * ntr:(t + 1) * ntr, :],
                                          in_=hloc[:ntr, t, :])

    return nc


# ---------------------------------------------------------------- host prep
_CACHE = {}
_last_in_maps = None


def _get_nc(n_nodes, nt, q):
    key = (n_nodes, nt, q)
    if key not in _CACHE:
        _apply_patches()
        nc = _build_nc(n_nodes, nt, q)
        _split_multiwaits(nc)
        _CACHE[key] = nc
    return _CACHE[key]


def _balance_tiles(degrees, nt, ntr):
    """Assign local node ids to nt tiles of exactly ntr nodes, balancing total
    edge count per tile."""
    order = np.argsort(-degrees, kind="stable")
    loads = np.zeros(nt, np.int64)
    counts = np.zeros(nt, np.int64)
    assign = np.empty(len(degrees), np.int64)
    for nid in order:
        open_bins = np.flatnonzero(counts < ntr)
        b = open_bins[np.argmin(loads[open_bins])]
        assign[nid] = b
        loads[b] += degrees[nid]
        counts[b] += 1
    return [np.flatnonzero(assign == t) for t in range(nt)]


def kernel(cg_xyz, CG_nbr_list, mapping, H,
           msg_w1, msg_b1, msg_w2, msg_b2, rbf_w, rbf_b,
           upd_wu, upd_wv, upd_w1, upd_b1, upd_w2, upd_b2,
           dense_w1, dense_b1, dense_w2, dense_b2):
    global _last_in_maps
    xyz = np.asarray(cg_xyz, np.float32)
    nbr = np.asarray(CG_nbr_list)
    H0 = np.asarray(H, np.float32)
    n_nodes = H0.shape[0]
    nsh = n_nodes // NCORE
    nt = max(1, (nsh + P - 1) // P)
    while nsh % nt:
        nt += 1
    ntr = nsh // nt

    ii = np.asarray(nbr[:, 0], np.int64)
    jj = np.asarray(nbr[:, 1], np.int64)
    r = xyz[jj] - xyz[ii]
    dist = np.sqrt((r.astype(np.float32) ** 2 + EPS).sum(-1))
    keep = dist < CUTOFF
    ii, jj, r, dist = ii[keep], jj[keep], r[keep], dist[keep]
    env = 0.5 * (np.cos(np.pi * dist / CUTOFF) + 1.0)
    unit = r / dist[:, None]
    nvec = np.arange(1, NRBF + 1, dtype=np.float32)
    rbf = np.sin(nvec[None, :] * np.pi * dist[:, None] / CUTOFF) / dist[:, None]
    rbf_e = np.concatenate([rbf * env[:, None], env[:, None]], 1)  # [E, 21]

    core = ii // nsh
    percore = []
    q = 1
    for k in range(NCORE):
        m = core == k
        il = (ii[m] - k * nsh).astype(np.int64)
        deg = np.bincount(il, minlength=nsh)
        tiles = _balance_tiles(deg, nt, ntr)
        tile_of = np.empty(nsh, np.int64)
        pos_of = np.empty(nsh, np.int64)
        for t, nodes in enumerate(tiles):
            tile_of[nodes] = t
            pos_of[nodes] = np.arange(len(nodes))
        et = tile_of[il]
        order = np.argsort(et, kind="stable")
        dat = dict(et=et[order], ilocp=pos_of[il][order], j=jj[m][order],
                   unit=unit[m][order], rbf=rbf_e[m][order])
        cnt = np.bincount(dat["et"], minlength=nt)
        q = max(q, int(np.ceil(cnt.max() / P)) if len(cnt) and cnt.max() else 1)
        percore.append((dat, cnt, tiles))

    ncols = nt * q
    s_slots = ncols * P
    bf16 = ml_dtypes.bfloat16

    def wslice(w):
        return np.ascontiguousarray(np.transpose(np.asarray(w, np.float32),
                                                 (1, 0, 2))).astype(bf16)

    rbf_wb = np.concatenate([np.asarray(rbf_w, np.float32),
                             np.asarray(rbf_b, np.float32)[:, None, :]], 1)
    wcom = {
        "w1h": wslice(msg_w1), "b1h": wslice(np.asarray(msg_b1)[:, None, :]),
        "w2h": wslice(msg_w2), "b2h": wslice(np.asarray(msg_b2)[:, None, :]),
        "rwh": wslice(rbf_wb),
        "wuh": wslice(upd_wu), "wvh": wslice(upd_wv),
        "u1a": wslice(np.asarray(upd_w1)[:, 0:F, :]),
        "u1b": wslice(np.asarray(upd_w1)[:, F:2 * F, :]),
        "ub1": wslice(np.asarray(upd_b1)[:, None, :]),
        "u2h": wslice(upd_w2), "ub2": wslice(np.asarray(upd_b2)[:, None, :]),
        "dw1": np.asarray(dense_w1, np.float32).astype(bf16),
        "db1": np.asarray(dense_b1, np.float32)[None, :].astype(bf16),
        "dw2": np.asarray(dense_w2, np.float32).astype(bf16),
        "db2": np.asarray(dense_b2, np.float32)[None, :].astype(bf16),
        "onesr": np.ones((1, P), bf16),
        "iota": np.tile(np.arange(P, dtype=np.float32), (P, 1)).astype(bf16),
        "ident": np.eye(P, dtype=np.float32).astype(bf16),
    }

    # global gather index: node g at (core k, tile t, pos p) -> k*nsh + t*ntr + p
    slot_of_global = np.empty(n_nodes, np.int64)
    for k in range(NCORE):
        _, _, tiles = percore[k]
        for t, nodes in enumerate(tiles):
            slot_of_global[k * nsh + nodes] = k * nsh + t * ntr + np.arange(len(nodes))

    in_maps = []
    node_order = []
    for k in range(NCORE):
        dat, cnt, tiles = percore[k]
        idx_s = np.zeros(s_slots, np.int32)
        iloc_s = np.full(s_slots, 200.0, np.float32)
        unit_s = np.zeros((s_slots, 3), np.float32)
        rbf_s = np.zeros((s_slots, NRB1), np.float32)
        pos = 0
        for t in range(nt):
            n_e = int(cnt[t])
            sl = slice(t * q * P, t * q * P + n_e)
            idx_s[sl] = slot_of_global[dat["j"][pos:pos + n_e]]
            iloc_s[sl] = dat["ilocp"][pos:pos + n_e]
            unit_s[sl] = dat["unit"][pos:pos + n_e]
            rbf_s[sl] = dat["rbf"][pos:pos + n_e]
            pos += n_e

        hshard = np.zeros((P, nt, F), np.float32)
        node_order.append(np.concatenate([k * nsh + nodes for nodes in tiles]))
        for t, nodes in enumerate(tiles):
            hshard[:len(nodes), t, :] = H0[k * nsh + nodes]

        m = dict(wcom)
        m["idxj"] = np.ascontiguousarray(idx_s.reshape(ncols, P).T)
        m["iloc"] = np.ascontiguousarray(iloc_s.reshape(ncols, P).T).astype(bf16)
        m["rbft"] = np.ascontiguousarray(rbf_s.T).astype(bf16)
        m["unitc"] = np.ascontiguousarray(
            unit_s.reshape(ncols, P, 3).transpose(1, 0, 2).reshape(P, ncols * 3))
        m["hsh"] = hshard
        in_maps.append(m)

    _last_in_maps = in_maps
    nc = _get_nc(n_nodes, nt, q)
    res = run_bass_kernel_spmd(nc, in_maps, core_ids=list(range(NCORE)))

    h_full = np.empty((n_nodes, F), np.float32)
    y_full = np.empty((n_nodes, 39), np.float32)
    for k in range(NCORE):
        h_full[node_order[k]] = res.results[k]["h_out"][:nsh]
        y_full[node_order[k]] = res.results[k]["y_out"][:nsh]
    return h_full, y_full.reshape(-1, 13, 3)
